# revision 8
# baseline (speedup 1.0000x reference)
"""Trainium2 Bass kernel for the gnn_message_passing block (8 NeuronCores).

Strategy (per core c, owning 512 global rows r = c*512..(c+1)*512):
  - Host rotates x_node/x_edge rows by -r0 so the owned rows sit first on
    every core (SPMD: one program, per-core data).
  - Associativity: mat @ (x @ W.T) == (mat @ x) @ W.T, so the five big
    N x N aggregations (adj@h shared by modules 0/4, four proj@k inputs)
    are computed ONCE per core as row-blocks (mat[r].T streamed from DRAM
    as the matmul moving operand; h/e tiles as stationary), producing
    feature-major outputs that feed the 512x512 projections directly.
  - rmsnorm weight vectors and the 1/sqrt(D) score scale are folded into
    the projection weights host-side; on-chip rmsnorm is the pure
    x * rsqrt(mean(x^2)+eps) form.
  - Per-node 8-head SDPA runs on DVE in fp16 (2x mode) with broadcast-AP
    multiplies and halving-tree + segmented reduces; exp on ACT.
  - FFNs: feature-major matmuls, gelu(+bias) fused on ACT, bias2 added
    via a K=1 ones matmul into PSUM.
Matmuls use float32r (full-rate fp32 on TRN2 when moving dim >= 256).
DMA loads are batched (one descriptor-heavy SWDGE start per 2-8 MiB) to
keep the Pool engine free.
"""
import numpy as np

N = 4096
E = 512
H = 8
D = 64
FF = 2048
P = 128
NCORES = 8
RPC = N // NCORES  # 512 rows per core
NT = N // P        # 32 tiles over all nodes
LT = RPC // P      # 4 local tiles
EPS = float(np.finfo(np.float32).eps)

_PROGRAM_CACHE = {}


def _split_big_waits(nc, mybir):
    """walrus in this toolchain rejects multi-wait instructions; cap at 1
    (2 for EventSemaphore), chaining the excess as EventSemaphores."""
    for f in nc.m.functions:
        for bb in f.blocks:
            insts = list(bb.instructions)
            out = []
            changed = False
            for inst in insts:
                si = inst.sync_info
                waits = list(si.on_wait) if si and si.on_wait else []
                cap = 2 if isinstance(inst, mybir.InstEventSemaphore) else 1
                if len(waits) > cap:
                    extra, keep = waits[:-cap], waits[-cap:]
                    for ci in range(0, len(extra), 2):
                        ev = mybir.InstEventSemaphore(name=f"{inst.name}-evw{ci}")
                        ev.engine = inst.engine
                        ev.sync_info = mybir.SyncInfo(on_wait=extra[ci:ci + 2],
                                                      on_update=[])
                        out.append(ev)
                    si.on_wait = keep
                    changed = True
                out.append(inst)
            if changed:
                bb.instructions[:] = out


def _build_program():
    import concourse.bass as bass
    import concourse.tile as tile
    from concourse import mybir
    from concourse.masks import make_identity
    from contextlib import ExitStack

    f32 = mybir.dt.float32
    f32r = mybir.dt.float32r
    f16 = mybir.dt.float16
    AF = mybir.ActivationFunctionType
    OP = mybir.AluOpType
    AX = mybir.AxisListType

    def bc(t, dims, off=0):
        """Strided/broadcast view of tile AP t; dims = (step, count) free
        axes; off = extra element offset."""
        return bass.AP(tensor=t.tensor, offset=t.offset + off,
                       ap=[list(t.ap[0])] + [[s, c] for (s, c) in dims])

    nc = bass.Bass()

    xn_d = nc.declare_dram_parameter("xn", [N, E], f32, isOutput=False)
    xe_d = nc.declare_dram_parameter("xe", [N, E], f32, isOutput=False)
    mat_d = [nc.declare_dram_parameter(f"mat{i}", [N, RPC], f32, isOutput=False)
             for i in range(5)]
    wq_d = nc.declare_dram_parameter("wqT", [H, E, E], f32, isOutput=False)
    wk_d = nc.declare_dram_parameter("wkT", [H, E, E], f32, isOutput=False)
    wv_d = nc.declare_dram_parameter("wvT", [H, E, E], f32, isOutput=False)
    w1h_d = nc.declare_dram_parameter("w1hT", [E, FF], f32, isOutput=False)
    w2h_d = nc.declare_dram_parameter("w2hT", [FF, E], f32, isOutput=False)
    w1e_d = nc.declare_dram_parameter("w1eT", [E, FF], f32, isOutput=False)
    w2e_d = nc.declare_dram_parameter("w2eT", [FF, E], f32, isOutput=False)
    b1h_d = nc.declare_dram_parameter("b1h", [FF], f32, isOutput=False)
    b2h_d = nc.declare_dram_parameter("b2h", [E], f32, isOutput=False)
    b1e_d = nc.declare_dram_parameter("b1e", [FF], f32, isOutput=False)
    b2e_d = nc.declare_dram_parameter("b2e", [E], f32, isOutput=False)
    outh_d = nc.declare_dram_parameter("outh", [RPC, E], f32, isOutput=True)
    oute_d = nc.declare_dram_parameter("oute", [RPC, E], f32, isOutput=True)

    with tile.TileContext(nc) as tc, ExitStack() as ctx:
        consts = ctx.enter_context(tc.tile_pool(name="consts", bufs=1))
        ident = consts.tile([P, P], f32)
        make_identity(nc, ident)
        ones1f = consts.tile([1, P], f32)
        nc.gpsimd.memset(ones1f, 1.0)
        ones1 = consts.tile([1, P], f32r)
        nc.scalar.copy(ones1[:], ones1f[:])
        eps_t = consts.tile([P, 1], f32)
        nc.vector.memset(eps_t, EPS)
        b1h_t = consts.tile([P, FF // P], f32)
        nc.gpsimd.dma_start(out=b1h_t, in_=b1h_d[:].rearrange("(c p) -> p c", p=P))
        b1e_t = consts.tile([P, FF // P], f32)
        nc.gpsimd.dma_start(out=b1e_t, in_=b1e_d[:].rearrange("(c p) -> p c", p=P))
        b2h_t = consts.tile([1, E], f32r)
        nc.gpsimd.dma_start(out=b2h_t, in_=b2h_d[:].rearrange("(a e) -> a e", a=1))
        b2e_t = consts.tile([1, E], f32r)
        nc.gpsimd.dma_start(out=b2e_t, in_=b2e_d[:].rearrange("(a e) -> a e", a=1))

        # long-lived pools
        aggp = ctx.enter_context(tc.tile_pool(name="agg", bufs=1))
        locp = ctx.enter_context(tc.tile_pool(name="loc", bufs=1))
        attp = ctx.enter_context(tc.tile_pool(name="att", bufs=1))
        statp = ctx.enter_context(tc.tile_pool(name="stat", bufs=4))

        aggT = [[None] * 4 for _ in range(5)]   # feature-major [128f, 512n]
        hTl = [locp.tile([P, RPC], f32r, tag=f"hTl{fc}", name=f"hTl{fc}")
               for fc in range(4)]
        eTl = [locp.tile([P, RPC], f32r, tag=f"eTl{fc}", name=f"eTl{fc}")
               for fc in range(4)]
        xatt_h = [attp.tile([P, E], f32, tag=f"xh{t}", name=f"xh{t}")
                  for t in range(LT)]
        xatt_e = [attp.tile([P, E], f32, tag=f"xe{t}", name=f"xe{t}")
                  for t in range(LT)]

        def rmsnorm_tile(dst, src_ap):
            """dst[128, 512] = pure rmsnorm of node-major slice."""
            stats = statp.tile([P, 6], f32, tag="bst", name="bst")
            nc.vector.bn_stats(out=stats[:], in_=src_ap)
            mv = statp.tile([P, 2], f32, tag="bag", name="bag")
            nc.vector.bn_aggr(out=mv[:], in_=stats[:])
            m2 = statp.tile([P, 1], f32, tag="m2", name="m2")
            nc.vector.tensor_tensor(out=m2[:], in0=mv[:, 0:1], in1=mv[:, 0:1],
                                    op=OP.mult)
            nc.vector.tensor_tensor(out=m2[:], in0=m2[:], in1=mv[:, 1:2], op=OP.add)
            sq = statp.tile([P, 1], f32, tag="sq", name="sq")
            nc.scalar.activation(out=sq[:], in_=m2[:], func=AF.Sqrt,
                                 bias=eps_t[:], scale=1.0)
            rs = statp.tile([P, 1], f32, tag="rs", name="rs")
            nc.vector.reciprocal(out=rs[:], in_=sq[:])
            nc.scalar.activation(out=dst[:], in_=src_ap, func=AF.Copy, scale=rs[:])

        def transpose_local(srcs, dstT, psp):
            for fc in range(4):
                ps = psp.tile([P, RPC], f32, tag="trps", bufs=2, name="trps")
                for t in range(4):
                    nc.tensor.transpose(ps[:, t * P:(t + 1) * P],
                                        srcs[t][:, fc * P:(fc + 1) * P].bitcast(f32),
                                        ident[:])
                nc.scalar.copy(dstT[fc][:], ps[:])

        def load_norm(x_dram, dst_tiles, xpool):
            """Stream x in 4 big DMAs; rmsnorm each 128-row tile."""
            for g in range(4):
                xg = xpool.tile([P, 8 * E], f32, tag="xing", name="xing")
                nc.sync.dma_start(
                    out=xg.rearrange("p (t e) -> p t e", e=E),
                    in_=x_dram[g * 8 * P:(g + 1) * 8 * P, :].rearrange(
                        "(t p) e -> p t e", p=P))
                for t in range(8):
                    rmsnorm_tile(dst_tiles[g * 8 + t], xg[:, t * E:(t + 1) * E])

        def aggregate(mi, lhs_tiles, psp, mpool):
            """aggT[mi][b] = feature-major block of (mat_mi @ x)."""
            pss = [psp.tile([P, E], f32, tag=f"agps{b}", name=f"agps{b}")
                   for b in range(4)]
            for g in range(4):
                mt = mpool.tile([P, 8 * RPC], f32r, tag="matg", bufs=2, name="matg")
                nc.gpsimd.dma_start(
                    out=mt.rearrange("p (t e) -> p t e", e=RPC),
                    in_=mat_d[mi][g * 8 * P:(g + 1) * 8 * P, :].rearrange(
                        "(t p) e -> p t e", p=P))
                for t in range(8):
                    ti = g * 8 + t
                    for b in range(4):
                        nc.tensor.matmul(
                            pss[b][:],
                            lhsT=lhs_tiles[ti][:, b * P:(b + 1) * P],
                            rhs=mt[:, t * RPC:(t + 1) * RPC],
                            start=(ti == 0), stop=(ti == NT - 1))
            for b in range(4):
                at = aggp.tile([P, E], f32r, tag=f"agg{mi}_{b}",
                               name=f"agg{mi}_{b}")
                nc.scalar.copy(at[:], pss[b][:])
                aggT[mi][b] = at

        # ---- phase 1a/2a: h, local transpose, h-based aggregations ----
        with tc.tile_pool(name="hfull", bufs=1) as hfp, \
             tc.tile_pool(name="xin_a", bufs=2) as xpool_a, \
             tc.tile_pool(name="ps_a", bufs=1, space="PSUM") as psp_a, \
             tc.tile_pool(name="mat_a", bufs=1) as mpool_a:
            h_sb = [hfp.tile([P, E], f32r, tag=f"h{t}", name=f"hsb{t}")
                    for t in range(NT)]
            load_norm(xn_d, h_sb, xpool_a)
            transpose_local(h_sb[:4], hTl, psp_a)
            for mi in (0, 1, 2):
                aggregate(mi, h_sb, psp_a, mpool_a)

        # ---- phase 1b/2b: e, local transpose, e-based aggregations ----
        with tc.tile_pool(name="efull", bufs=1) as efp, \
             tc.tile_pool(name="xin_b", bufs=2) as xpool_b, \
             tc.tile_pool(name="ps_b", bufs=1, space="PSUM") as psp_b, \
             tc.tile_pool(name="mat_b", bufs=1) as mpool_b:
            e_sb = [efp.tile([P, E], f32r, tag=f"e{t}", name=f"esb{t}")
                    for t in range(NT)]
            load_norm(xe_d, e_sb, xpool_b)
            transpose_local(e_sb[:4], eTl, psp_b)
            for mi in (3, 4):
                aggregate(mi, e_sb, psp_b, mpool_b)

        # ---- phase 3: the 8 attention modules ----
        def m_inputs(m):
            qsrc = {0: aggT[0], 1: eTl, 2: eTl, 3: hTl,
                    4: aggT[0], 5: eTl, 6: eTl, 7: hTl}[m]
            ksrc = {0: hTl, 1: eTl, 2: aggT[1], 3: aggT[3],
                    4: hTl, 5: eTl, 6: aggT[2], 7: aggT[4]}[m]
            return qsrc, ksrc

        MORDER = [0, 4, 2, 3, 6, 7, 1, 5]

        with tc.tile_pool(name="wts", bufs=2) as wpool, \
             tc.tile_pool(name="qkv", bufs=2) as qkvp, \
             tc.tile_pool(name="sdtmp", bufs=1) as tmpp, \
             tc.tile_pool(name="sdsm", bufs=4) as smp, \
             tc.tile_pool(name="ps_c", bufs=1, space="PSUM") as psp_c:
            for m in MORDER:
                qsrc, ksrc = m_inputs(m)
                branch_att = xatt_h if m < 4 else xatt_e
                w_ts = {}
                for (dram, nm) in ((wq_d, "wq"), (wk_d, "wk"), (wv_d, "wv")):
                    wt = wpool.tile([P, 4 * E], f32r, tag=nm, name=f"w_{nm}")
                    nc.gpsimd.dma_start(
                        out=wt.rearrange("p (fc e) -> p fc e", e=E),
                        in_=dram[m].rearrange("(fc p) e -> p fc e", p=P))
                    w_ts[nm] = wt

                q_sb, k_sb, v_sb = [], [], []
                for (srcT, wnm, lst) in ((qsrc, "wq", q_sb), (ksrc, "wk", k_sb),
                                         (hTl, "wv", v_sb)):
                    wt = w_ts[wnm]
                    for b in range(LT):
                        ps = psp_c.tile([P, E], f32, tag="projps", bufs=4,
                                        name="projps")
                        for fc in range(4):
                            nc.tensor.matmul(
                                ps[:],
                                lhsT=srcT[fc][:, b * P:(b + 1) * P],
                                rhs=wt[:, fc * E:(fc + 1) * E],
                                start=(fc == 0), stop=(fc == 3))
                        dt = qkvp.tile([P, E], f16, tag=f"{wnm}_{b}",
                                       name=f"qkv{b}")
                        if wnm == "wv":
                            # write v as [n, d*8+g] so g is innermost later
                            nc.scalar.copy(bc(dt, [(1, 8), (8, 64)]), ps[:])
                        else:
                            nc.scalar.copy(dt[:], ps[:])
                        lst.append(dt)

                for t in range(LT):
                    q_t, k_t, v_t = q_sb[t], k_sb[t], v_sb[t]
                    # scores: tmp[p, h, g, d] = q[p,h,d] * k[p,g,d]  (fp16 2x)
                    tmp = tmpp.tile([P, H * H * D], f16, tag="sdpa", bufs=3,
                                    name="sdpa")
                    nc.vector.tensor_tensor(
                        out=bc(tmp, [(512, 8), (64, 8), (1, 64)]),
                        in0=bc(q_t, [(64, 8), (0, 8), (1, 64)]),
                        in1=bc(k_t, [(0, 8), (64, 8), (1, 64)]),
                        op=OP.mult)
                    # halving tree over d: 64 -> 8 slots
                    for dd in (32, 16, 8):
                        nc.vector.tensor_tensor(
                            out=bc(tmp, [(64, 64), (1, dd)]),
                            in0=bc(tmp, [(64, 64), (1, dd)]),
                            in1=bc(tmp, [(64, 64), (1, dd)], off=dd),
                            op=OP.add)
                    s_t = smp.tile([P, H * H], f32, tag="s", name="s")
                    nc.vector.tensor_reduce(
                        out=s_t[:], in_=bc(tmp, [(64, 64), (1, 8)]),
                        axis=AX.X, op=OP.add)
                    ex_t = smp.tile([P, H * H], f16, tag="ex", name="ex")
                    nc.scalar.activation(out=ex_t[:], in_=s_t[:], func=AF.Exp)
                    den = smp.tile([P, H], f32, tag="den", name="den")
                    nc.vector.tensor_reduce(
                        out=den[:], in_=ex_t.rearrange("p (h g) -> p h g", g=H),
                        axis=AX.X, op=OP.add)
                    rden = smp.tile([P, H], f32, tag="rden", name="rden")
                    nc.vector.reciprocal(out=rden[:], in_=den[:])
                    a_t = smp.tile([P, H * H], f16, tag="a", name="a")
                    nc.vector.tensor_tensor(
                        out=bc(a_t, [(8, 8), (1, 8)]),
                        in0=bc(ex_t, [(8, 8), (1, 8)]),
                        in1=bc(rden, [(1, 8), (0, 8)]),
                        op=OP.mult)
                    # attn out: tmp2[p, h, d, g] = a[p,h,g] * v[p,g,d]
                    tmp2 = tmpp.tile([P, H * H * D], f16, tag="sdpa", bufs=3,
                                     name="sdpa2")
                    nc.vector.tensor_tensor(
                        out=bc(tmp2, [(512, 8), (8, 64), (1, 8)]),
                        in0=bc(a_t, [(8, 8), (0, 64), (1, 8)]),
                        in1=bc(v_t, [(0, 8), (8, 64), (1, 8)]),
                        op=OP.mult)
                    # halving tree over g: 8 -> 2, then final f32 add
                    for gg in (4, 2):
                        nc.vector.tensor_tensor(
                            out=bc(tmp2, [(8, 512), (1, gg)]),
                            in0=bc(tmp2, [(8, 512), (1, gg)]),
                            in1=bc(tmp2, [(8, 512), (1, gg)], off=gg),
                            op=OP.add)
                    if m in (0, 4):
                        nc.vector.tensor_tensor(
                            out=branch_att[t][:],
                            in0=bc(tmp2, [(8, 512)]),
                            in1=bc(tmp2, [(8, 512)], off=1),
                            op=OP.add)
                    else:
                        rt = smp.tile([P, E], f32, tag="avred", name="avred")
                        nc.vector.tensor_tensor(
                            out=rt[:],
                            in0=bc(tmp2, [(8, 512)]),
                            in1=bc(tmp2, [(8, 512)], off=1),
                            op=OP.add)
                        nc.vector.tensor_tensor(out=branch_att[t][:],
                                                in0=branch_att[t][:], in1=rt[:],
                                                op=OP.add)

        # ---- phase 4: rms2 + FFN per branch ----
        def ffn(branch_att, w1_dram, w2_dram, b1_t, b2_t, out_dram):
            with tc.tile_pool(name="ffn_sb", bufs=1) as fsb, \
                 tc.tile_pool(name="ffn_xn", bufs=1) as fxn, \
                 tc.tile_pool(name="ffn_ps", bufs=1, space="PSUM") as fps, \
                 tc.tile_pool(name="ffn_out", bufs=1) as fout:
                xn_tiles = []
                for t in range(LT):
                    xt = fxn.tile([P, E], f32, tag=f"fx{t}", name=f"fx{t}")
                    rmsnorm_tile(xt, branch_att[t][:])
                    xn_tiles.append(xt)
                xnT = []
                for fc in range(4):
                    ps = fps.tile([P, RPC], f32, tag="ftr", bufs=2, name="ftr")
                    for t in range(4):
                        nc.tensor.transpose(ps[:, t * P:(t + 1) * P],
                                            xn_tiles[t][:, fc * P:(fc + 1) * P],
                                            ident[:])
                    xt = fxn.tile([P, RPC], f32r, tag=f"fxT{fc}", name=f"fxT{fc}")
                    nc.scalar.copy(xt[:], ps[:])
                    xnT.append(xt)
                w1_t = fsb.tile([P, 4 * FF], f32r, tag="w1", name="w1")
                nc.gpsimd.dma_start(
                    out=w1_t.rearrange("p (fc e) -> p fc e", e=FF),
                    in_=w1_dram[:, :].rearrange("(fc p) e -> p fc e", p=P))
                g1 = []
                for ffb in range(FF // P):
                    ps = fps.tile([P, RPC], f32, tag="fps1", bufs=4, name="fps1")
                    for fc in range(4):
                        nc.tensor.matmul(
                            ps[:],
                            lhsT=w1_t[:, fc * FF + ffb * P:fc * FF + (ffb + 1) * P],
                            rhs=xnT[fc][:],
                            start=(fc == 0), stop=(fc == 3))
                    gt = fsb.tile([P, RPC], f32r, tag=f"g1_{ffb}", name=f"g1_{ffb}")
                    nc.scalar.activation(out=gt[:], in_=ps[:], func=AF.Gelu,
                                         bias=b1_t[:, ffb:ffb + 1], scale=1.0)
                    g1.append(gt)
                w2_t = fsb.tile([P, 16 * E], f32r, tag="w2", name="w2")
                nc.gpsimd.dma_start(
                    out=w2_t.rearrange("p (fc e) -> p fc e", e=E),
                    in_=w2_dram[:, :].rearrange("(fc p) e -> p fc e", p=P))
                ot = fout.tile([P, 4 * E], f32, tag="fo", name="fo")
                for b in range(LT):
                    ps = fps.tile([P, E], f32, tag="fps2", bufs=2, name="fps2")
                    for ffc in range(FF // P):
                        nc.tensor.matmul(
                            ps[:],
                            lhsT=g1[ffc][:, b * P:(b + 1) * P],
                            rhs=w2_t[:, ffc * E:(ffc + 1) * E],
                            start=(ffc == 0), stop=False)
                    nc.tensor.matmul(ps[:], lhsT=ones1[:], rhs=b2_t[:],
                                     start=False, stop=True)
                    nc.scalar.copy(ot[:, b * E:(b + 1) * E], ps[:])
                nc.sync.dma_start(
                    out=out_dram[:, :].rearrange("(b p) e -> p b e", p=P),
                    in_=ot.rearrange("p (b e) -> p b e", e=E))

        ffn(xatt_h, w1h_d, w2h_d, b1h_t, b2h_t, outh_d)
        ffn(xatt_e, w1e_d, w2e_d, b1e_t, b2e_t, oute_d)

    _split_big_waits(nc, mybir)
    return nc


def _get_program():
    if "nc" not in _PROGRAM_CACHE:
        _PROGRAM_CACHE["nc"] = _build_program()
    return _PROGRAM_CACHE["nc"]


def _prep_inputs(x_node, x_edge, adj, Wq, Wk, Wv,
                 proj_he_h, proj_eh_h, proj_he_e, proj_eh_e,
                 rms1_h, rms1_e, rms2_h,
                 w1_h, b1_h, w2_h, b2_h, w1_e, b1_e, w2_e, b2_e):
    """Per-core input dicts. Weight folding + row rotation happen here."""
    f = np.float32
    wsrc_q = [rms1_h, rms1_e, rms1_e, rms1_h, rms1_h, rms1_e, rms1_e, rms1_h]
    wsrc_k = [rms1_h, rms1_e, rms1_h, rms1_e, rms1_h, rms1_e, rms1_h, rms1_e]
    wqT = np.stack([(Wq[m].T * wsrc_q[m][:, None]) * 0.125 for m in range(H)])
    wkT = np.stack([Wk[m].T * wsrc_k[m][:, None] for m in range(H)])
    wvT = np.stack([Wv[m].T * rms1_h[:, None] for m in range(H)])
    w1hT = np.ascontiguousarray((w1_h * rms2_h[None, :]).T, dtype=f)
    w1eT = np.ascontiguousarray((w1_e * rms2_h[None, :]).T, dtype=f)
    w2hT = np.ascontiguousarray(w2_h.T, dtype=f)
    w2eT = np.ascontiguousarray(w2_e.T, dtype=f)
    mats = [adj, proj_eh_h, proj_eh_e, proj_he_h, proj_he_e]

    shared = dict(wqT=np.ascontiguousarray(wqT, f), wkT=np.ascontiguousarray(wkT, f),
                  wvT=np.ascontiguousarray(wvT, f),
                  w1hT=w1hT, w2hT=w2hT, w1eT=w1eT, w2eT=w2eT,
                  b1h=b1_h.astype(f), b2h=b2_h.astype(f),
                  b1e=b1_e.astype(f), b2e=b2_e.astype(f))
    in_maps = []
    for c in range(NCORES):
        r0 = c * RPC
        m = dict(shared)
        m["xn"] = np.ascontiguousarray(np.roll(x_node, -r0, axis=0), f)
        m["xe"] = np.ascontiguousarray(np.roll(x_edge, -r0, axis=0), f)
        for i, mat in enumerate(mats):
            mt = np.ascontiguousarray(mat[r0:r0 + RPC].T, f)  # [N, RPC]
            m[f"mat{i}"] = np.ascontiguousarray(np.roll(mt, -r0, axis=0))
        in_maps.append(m)
    return in_maps


def kernel(**inputs):
    from concourse.bass_utils import run_bass_kernel_spmd
    nc = _get_program()
    in_maps = _prep_inputs(**{k: np.asarray(v) for k, v in inputs.items()})
    res = run_bass_kernel_spmd(nc, in_maps, list(range(NCORES))).results
    x_h = np.concatenate([res[c]["outh"] for c in range(NCORES)], axis=0)
    x_e = np.concatenate([res[c]["oute"] for c in range(NCORES)], axis=0)
    return (x_h, x_e)


# revision 18
# speedup vs baseline: 7.9784x; 7.9784x over previous
"""Trainium2 Bass kernel for the gnn_message_passing block (8 NeuronCores).

Strategy (per core c, owning 512 global rows r = c*512..(c+1)*512):
  - Host rotates x_node/x_edge rows by -r0 so the owned rows sit first on
    every core (SPMD: one program, per-core data).
  - Associativity: mat @ (x @ W.T) == (mat @ x) @ W.T, so the five big
    N x N aggregations (adj@h shared by modules 0/4, four proj@k inputs)
    are computed ONCE per core as row-blocks (mat[r].T streamed from DRAM
    as the bf16 moving operand; bf16 h/e tiles stationary), producing
    feature-major f32r outputs that feed the 512x512 projections.
  - rmsnorm weight vectors and the 1/sqrt(D) score scale are folded into
    the projection weights host-side; on-chip rmsnorm is the pure
    x * rsqrt(mean(x^2)+eps) form, computed on ACT (square+accum).
  - Per-node 8-head SDPA runs on DVE in fp16 (2x mode) with broadcast-AP
    multiplies and halving-tree + segmented reduces; exp on ACT.
  - Module processing is interleaved with the aggregations so PE (matmuls)
    and DVE (SDPA) run concurrently:
      loads(h,e0-3) | mod 1,5 | agg0 | mod 0,4 | agg1,2 | loads(e) |
      agg3 | mod 3 | agg4 | mod 7 | mod 2 | FFN-h | mod 6 | FFN-e
  - FFNs: feature-major matmuls, gelu(+bias) on ACT, bias2 via K=1 matmul.
Projection/FFN matmuls use float32r (full-rate fp32, moving dim >= 256).
DMA loads are batched (4-8 tiles per SWDGE start) to keep Pool free.
"""
import numpy as np

N = 4096
E = 512
H = 8
D = 64
FF = 2048
P = 128
NCORES = 8
RPC = N // NCORES  # 512 rows per core
NT = N // P        # 32 tiles over all nodes
LT = RPC // P      # 4 local tiles
EPS = float(np.finfo(np.float32).eps)

_PROGRAM_CACHE = {}


def _split_big_waits(nc, mybir):
    """walrus in this toolchain rejects multi-wait instructions; cap at 1
    (2 for EventSemaphore), chaining the excess as EventSemaphores."""
    for f in nc.m.functions:
        for bb in f.blocks:
            insts = list(bb.instructions)
            out = []
            changed = False
            for inst in insts:
                si = inst.sync_info
                waits = list(si.on_wait) if si and si.on_wait else []
                cap = 2 if isinstance(inst, mybir.InstEventSemaphore) else 1
                if len(waits) > cap:
                    extra, keep = waits[:-cap], waits[-cap:]
                    for ci in range(0, len(extra), 2):
                        ev = mybir.InstEventSemaphore(name=f"{inst.name}-evw{ci}")
                        ev.engine = inst.engine
                        ev.sync_info = mybir.SyncInfo(on_wait=extra[ci:ci + 2],
                                                      on_update=[])
                        out.append(ev)
                    si.on_wait = keep
                    changed = True
                out.append(inst)
            if changed:
                bb.instructions[:] = out


def _build_program():
    import concourse.bass as bass
    import concourse.tile as tile
    from concourse import mybir
    from concourse.masks import make_identity
    from contextlib import ExitStack

    f32 = mybir.dt.float32
    f32r = mybir.dt.float32r
    f16 = mybir.dt.float16
    bf16 = mybir.dt.bfloat16
    AF = mybir.ActivationFunctionType
    OP = mybir.AluOpType
    AX = mybir.AxisListType

    def bc(t, dims, off=0):
        return bass.AP(tensor=t.tensor, offset=t.offset + off,
                       ap=[list(t.ap[0])] + [[s, c] for (s, c) in dims])

    nc = bass.Bass()

    xn_d = nc.declare_dram_parameter("xn", [N, E], f32, isOutput=False)
    xe_d = nc.declare_dram_parameter("xe", [N, E], f32, isOutput=False)
    mat_d = [nc.declare_dram_parameter(f"mat{i}", [N, RPC], f32, isOutput=False)
             for i in range(5)]
    wq_d = nc.declare_dram_parameter("wqT", [H, E, E], f32, isOutput=False)
    wk_d = nc.declare_dram_parameter("wkT", [H, E, E], f32, isOutput=False)
    wv_d = nc.declare_dram_parameter("wvT", [H, E, E], f32, isOutput=False)
    w1h_d = nc.declare_dram_parameter("w1hT", [E, FF], f32, isOutput=False)
    w2h_d = nc.declare_dram_parameter("w2hT", [FF, E], f32, isOutput=False)
    w1e_d = nc.declare_dram_parameter("w1eT", [E, FF], f32, isOutput=False)
    w2e_d = nc.declare_dram_parameter("w2eT", [FF, E], f32, isOutput=False)
    b1h_d = nc.declare_dram_parameter("b1h", [FF], f32, isOutput=False)
    b2h_d = nc.declare_dram_parameter("b2h", [E], f32, isOutput=False)
    b1e_d = nc.declare_dram_parameter("b1e", [FF], f32, isOutput=False)
    b2e_d = nc.declare_dram_parameter("b2e", [E], f32, isOutput=False)
    outh_d = nc.declare_dram_parameter("outh", [RPC, E], f32, isOutput=True)
    oute_d = nc.declare_dram_parameter("oute", [RPC, E], f32, isOutput=True)

    with tile.TileContext(nc, pool_alloc_mode="queue") as tc, ExitStack() as ctx:
        consts = ctx.enter_context(tc.tile_pool(name="consts", bufs=1))
        ident = consts.tile([P, P], f32)
        make_identity(nc, ident)
        ones1f = consts.tile([1, P], f32)
        nc.gpsimd.memset(ones1f, 1.0)
        ones1 = consts.tile([1, P], f32r)
        nc.scalar.copy(ones1[:], ones1f[:])
        eps_t = consts.tile([P, 1], f32)
        nc.vector.memset(eps_t, EPS)
        b1h_t = consts.tile([P, FF // P], f32)
        nc.gpsimd.dma_start(out=b1h_t, in_=b1h_d[:].rearrange("(c p) -> p c", p=P))
        b1e_t = consts.tile([P, FF // P], f32)
        nc.gpsimd.dma_start(out=b1e_t, in_=b1e_d[:].rearrange("(c p) -> p c", p=P))
        b2h_t = consts.tile([1, E], f32r)
        nc.gpsimd.dma_start(out=b2h_t, in_=b2h_d[:].rearrange("(a e) -> a e", a=1))
        b2e_t = consts.tile([1, E], f32r)
        nc.gpsimd.dma_start(out=b2e_t, in_=b2e_d[:].rearrange("(a e) -> a e", a=1))

        # whole-program pools
        locp = ctx.enter_context(tc.tile_pool(name="loc", bufs=1))
        attp = ctx.enter_context(tc.tile_pool(name="att", bufs=1))
        statp = ctx.enter_context(tc.tile_pool(name="stat", bufs=4))
        sqscp = ctx.enter_context(tc.tile_pool(name="sqsc", bufs=1))
        wpool = ctx.enter_context(tc.tile_pool(name="wts", bufs=1))
        qkvp = ctx.enter_context(tc.tile_pool(name="qkv", bufs=1))
        tmpp = ctx.enter_context(tc.tile_pool(name="sdtmp", bufs=1))
        smp = ctx.enter_context(tc.tile_pool(name="sdsm", bufs=2))
        psp = ctx.enter_context(tc.tile_pool(name="ps", bufs=1, space="PSUM"))

        hTl = [locp.tile([P, RPC], f32r, tag=f"hTl{fc}", name=f"hTl{fc}")
               for fc in range(4)]
        eTl = [locp.tile([P, RPC], f32r, tag=f"eTl{fc}", name=f"eTl{fc}")
               for fc in range(4)]
        xatt_h = [attp.tile([P, E], f32, tag=f"xh{t}", name=f"xh{t}")
                  for t in range(LT)]
        xatt_e = [attp.tile([P, E], f32, tag=f"xe{t}", name=f"xe{t}")
                  for t in range(LT)]

        def rmsnorm_tile(dst, src_ap, dst2=None):
            """dst = pure rmsnorm of node-major [128, 512] slice (ACT-heavy)."""
            sc = sqscp.tile([P, E], f32, tag="sqsc", name="sqsc")
            ssq = statp.tile([P, 1], f32, tag="ssq", name="ssq")
            nc.scalar.activation(out=sc[:], in_=src_ap, func=AF.Square,
                                 accum_out=ssq[:])
            sq = statp.tile([P, 1], f32, tag="sq", name="sq")
            nc.scalar.activation(out=sq[:], in_=ssq[:], func=AF.Sqrt,
                                 bias=eps_t[:], scale=1.0 / E)
            rs = statp.tile([P, 1], f32, tag="rs", name="rs")
            nc.vector.reciprocal(out=rs[:], in_=sq[:])
            nc.gpsimd.tensor_scalar_mul(dst[:], src_ap, rs[:])
            if dst2 is not None:
                nc.scalar.activation(out=dst2[:], in_=src_ap, func=AF.Copy,
                                     scale=rs[:])

        def load_norm(x_dram, dst_tiles, t0, t1, xpool, dst2=None):
            """Stream x rows [t0*128, t1*128) in 4-tile DMAs; rmsnorm each."""
            for g0 in range(t0, t1, 4):
                xg = xpool.tile([P, 4 * E], f32, tag="xing", name="xing")
                nc.sync.dma_start(
                    out=xg.rearrange("p (t e) -> p t e", e=E),
                    in_=x_dram[g0 * P:(g0 + 4) * P, :].rearrange(
                        "(t p) e -> p t e", p=P))
                for t in range(4):
                    ti = g0 + t
                    rmsnorm_tile(dst_tiles[ti], xg[:, t * E:(t + 1) * E],
                                 dst2=(dst2[ti] if dst2 and ti < 4 else None))

        def transpose_local(srcs, dstT):
            for fc in range(4):
                ps = psp.tile([P, RPC], f32, tag="projps", bufs=4, name="trps")
                for t in range(4):
                    nc.tensor.transpose(ps[:, t * P:(t + 1) * P],
                                        srcs[t][:, fc * P:(fc + 1) * P].bitcast(f32),
                                        ident[:])
                nc.scalar.copy(dstT[fc][:], ps[:])

        def aggregate(mi, lhs_tiles, aggpool):
            """returns 4 feature-major f32r [128, 512] blocks of mat_mi @ x."""
            mst = ExitStack()
            matgp = mst.enter_context(tc.tile_pool(name=f"matg{mi}", bufs=2))
            pss = [psp.tile([P, E], f32, tag=f"agps{b}", name=f"agps{b}")
                   for b in range(4)]
            for g in range(8):
                mt = matgp.tile([P, 4 * RPC], bf16, tag="matg", name="matg")
                nc.gpsimd.dma_start(
                    out=mt.rearrange("p (t e) -> p t e", e=RPC),
                    in_=mat_d[mi][g * 4 * P:(g + 1) * 4 * P, :].rearrange(
                        "(t p) e -> p t e", p=P))
                for t in range(4):
                    ti = g * 4 + t
                    for b in range(4):
                        nc.tensor.matmul(
                            pss[b][:],
                            lhsT=lhs_tiles[ti][:, b * P:(b + 1) * P],
                            rhs=mt[:, t * RPC:(t + 1) * RPC],
                            start=(ti == 0), stop=(ti == NT - 1))
            outt = []
            for b in range(4):
                at = aggpool.tile([P, E], f32r, tag=f"ag{mi}_{b}",
                                  name=f"ag{mi}_{b}")
                nc.scalar.copy(at[:], pss[b][:])
                outt.append(at)
            mst.close()
            return outt

        def module(m, qsrc, ksrc, branch_att, first):
            w_ts = {}
            for (dram, nm) in ((wq_d, "wq"), (wk_d, "wk"), (wv_d, "wv")):
                wt = wpool.tile([P, 4 * E], f32r, tag=nm, name=f"w_{nm}")
                nc.gpsimd.dma_start(
                    out=wt.rearrange("p (fc e) -> p fc e", e=E),
                    in_=dram[m].rearrange("(fc p) e -> p fc e", p=P))
                w_ts[nm] = wt

            q_sb, k_sb, v_sb = [], [], []
            for (srcT, wnm, lst) in ((qsrc, "wq", q_sb), (ksrc, "wk", k_sb),
                                     (hTl, "wv", v_sb)):
                wt = w_ts[wnm]
                for b in range(LT):
                    ps = psp.tile([P, E], f32, tag="projps", bufs=4,
                                  name="projps")
                    for fc in range(4):
                        nc.tensor.matmul(
                            ps[:],
                            lhsT=srcT[fc][:, b * P:(b + 1) * P],
                            rhs=wt[:, fc * E:(fc + 1) * E],
                            start=(fc == 0), stop=(fc == 3))
                    dt = qkvp.tile([P, E], f16, tag=f"{wnm}_{b}", name=f"qkv{b}")
                    if wnm == "wv":
                        nc.scalar.copy(bc(dt, [(1, 8), (8, 64)]), ps[:])
                    else:
                        nc.scalar.copy(dt[:], ps[:])
                    lst.append(dt)

            for t in range(LT):
                q_t, k_t, v_t = q_sb[t], k_sb[t], v_sb[t]
                tmp = tmpp.tile([P, H * H * D], f16, tag="sdpa", bufs=1,
                                name="sdpa")
                nc.vector.tensor_tensor(
                    out=bc(tmp, [(512, 8), (64, 8), (1, 64)]),
                    in0=bc(q_t, [(64, 8), (0, 8), (1, 64)]),
                    in1=bc(k_t, [(0, 8), (64, 8), (1, 64)]),
                    op=OP.mult)
                for dd in (32, 16, 8):
                    nc.vector.tensor_tensor(
                        out=bc(tmp, [(64, 64), (1, dd)]),
                        in0=bc(tmp, [(64, 64), (1, dd)]),
                        in1=bc(tmp, [(64, 64), (1, dd)], off=dd),
                        op=OP.add)
                s_t = smp.tile([P, H * H], f32, tag="s", name="s")
                nc.vector.tensor_reduce(
                    out=s_t[:], in_=bc(tmp, [(64, 64), (1, 8)]),
                    axis=AX.X, op=OP.add)
                ex_t = smp.tile([P, H * H], f16, tag="ex", name="ex")
                nc.scalar.activation(out=ex_t[:], in_=s_t[:], func=AF.Exp)
                den = smp.tile([P, H], f32, tag="den", name="den")
                nc.vector.tensor_reduce(
                    out=den[:], in_=ex_t.rearrange("p (h g) -> p h g", g=H),
                    axis=AX.X, op=OP.add)
                rden = smp.tile([P, H], f32, tag="rden", name="rden")
                nc.vector.reciprocal(out=rden[:], in_=den[:])
                a_t = smp.tile([P, H * H], f16, tag="a", name="a")
                nc.vector.tensor_tensor(
                    out=bc(a_t, [(8, 8), (1, 8)]),
                    in0=bc(ex_t, [(8, 8), (1, 8)]),
                    in1=bc(rden, [(1, 8), (0, 8)]),
                    op=OP.mult)
                tmp2 = tmpp.tile([P, H * H * D], f16, tag="sdpa", bufs=1,
                                 name="sdpa2")
                nc.vector.tensor_tensor(
                    out=bc(tmp2, [(512, 8), (8, 64), (1, 8)]),
                    in0=bc(a_t, [(8, 8), (0, 64), (1, 8)]),
                    in1=bc(v_t, [(0, 8), (8, 64), (1, 8)]),
                    op=OP.mult)
                for gg in (4, 2):
                    nc.vector.tensor_tensor(
                        out=bc(tmp2, [(8, 512), (1, gg)]),
                        in0=bc(tmp2, [(8, 512), (1, gg)]),
                        in1=bc(tmp2, [(8, 512), (1, gg)], off=gg),
                        op=OP.add)
                if first:
                    nc.vector.tensor_tensor(
                        out=branch_att[t][:],
                        in0=bc(tmp2, [(8, 512)]),
                        in1=bc(tmp2, [(8, 512)], off=1),
                        op=OP.add)
                else:
                    rt = smp.tile([P, E], f32, tag="avred", name="avred")
                    nc.vector.tensor_tensor(
                        out=rt[:],
                        in0=bc(tmp2, [(8, 512)]),
                        in1=bc(tmp2, [(8, 512)], off=1),
                        op=OP.add)
                    nc.gpsimd.tensor_tensor(out=branch_att[t][:],
                                            in0=branch_att[t][:], in1=rt[:],
                                            op=OP.add)

        def ffn(branch_att, w1_dram, w2_dram, b1_t, b2_t, out_dram):
            with tc.tile_pool(name="ffn_sb", bufs=1) as fsb, \
                 tc.tile_pool(name="ffn_xn", bufs=1) as fxn:
                xn_tiles = []
                for t in range(LT):
                    xt = fxn.tile([P, E], f32, tag=f"fx{t}", name=f"fx{t}")
                    rmsnorm_tile(xt, branch_att[t][:])
                    xn_tiles.append(xt)
                xnT = []
                for fc in range(4):
                    ps = psp.tile([P, RPC], f32, tag="agps0", name="ftr")
                    for t in range(4):
                        nc.tensor.transpose(ps[:, t * P:(t + 1) * P],
                                            xn_tiles[t][:, fc * P:(fc + 1) * P],
                                            ident[:])
                    xt = fxn.tile([P, RPC], f32r, tag=f"fxT{fc}", name=f"fxT{fc}")
                    nc.scalar.copy(xt[:], ps[:])
                    xnT.append(xt)
                g1 = []
                HW1 = FF // 2
                for half in range(2):
                    w1_t = fsb.tile([P, 4 * HW1], f32r, tag="w1", name="w1")
                    nc.gpsimd.dma_start(
                        out=w1_t.rearrange("p (fc e) -> p fc e", e=HW1),
                        in_=w1_dram[:, half * HW1:(half + 1) * HW1].rearrange(
                            "(fc p) e -> p fc e", p=P))
                    for fb in range(HW1 // P):
                        ffb = half * (HW1 // P) + fb
                        ps = psp.tile([P, RPC], f32, tag=f"agps{1 + ffb % 2}",
                                      name="fps1")
                        for fc in range(4):
                            nc.tensor.matmul(
                                ps[:],
                                lhsT=w1_t[:, fc * HW1 + fb * P:
                                          fc * HW1 + (fb + 1) * P],
                                rhs=xnT[fc][:],
                                start=(fc == 0), stop=(fc == 3))
                        gt = fsb.tile([P, RPC], f16, tag=f"g1_{ffb}",
                                      name=f"g1_{ffb}")
                        nc.scalar.activation(out=gt[:], in_=ps[:], func=AF.Gelu,
                                             bias=b1_t[:, ffb:ffb + 1], scale=1.0)
                        g1.append(gt)
                w2_t = fsb.tile([P, 16 * E], f16, tag="w2", name="w2")
                nc.gpsimd.dma_start(
                    out=w2_t.rearrange("p (fc e) -> p fc e", e=E),
                    in_=w2_dram[:, :].rearrange("(fc p) e -> p fc e", p=P))
                ot = fsb.tile([P, 4 * E], f32, tag="fo", name="fo")
                for b in range(LT):
                    ps = psp.tile([P, E], f32, tag="agps3", name="fps2")
                    for ffc in range(FF // P):
                        nc.tensor.matmul(
                            ps[:],
                            lhsT=g1[ffc][:, b * P:(b + 1) * P],
                            rhs=w2_t[:, ffc * E:(ffc + 1) * E],
                            start=(ffc == 0), stop=False)
                    nc.tensor.matmul(ps[:], lhsT=ones1[:], rhs=b2_t[:],
                                     start=False, stop=True)
                    nc.scalar.copy(ot[:, b * E:(b + 1) * E], ps[:])
                nc.sync.dma_start(
                    out=out_dram[:, :].rearrange("(b p) e -> p b e", p=P),
                    in_=ot.rearrange("p (b e) -> p b e", e=E))

        # ======== emission order (the schedule) ========
        # Pool open/close must be LIFO: agglate (aggs 1-4) and eearly
        # outlive hfull; agg0/erest nest inside.
        aggl12_stack = ExitStack()
        agglate12 = aggl12_stack.enter_context(
            tc.tile_pool(name="agglate12", bufs=1))
        aggl34_stack = ExitStack()
        agglate34 = aggl34_stack.enter_context(
            tc.tile_pool(name="agglate34", bufs=1))
        eearly_stack = ExitStack()
        eearly = eearly_stack.enter_context(tc.tile_pool(name="eearly", bufs=1))
        hstack = ExitStack()
        hfp = hstack.enter_context(tc.tile_pool(name="hfull", bufs=1))

        h_sb = [hfp.tile([P, E], bf16, tag=f"h{t}", name=f"hsb{t}")
                for t in range(NT)]
        e_sb = [eearly.tile([P, E], bf16, tag=f"e{t}", name=f"esb{t}")
                for t in range(4)]
        x04_stack = ExitStack()
        x04p = x04_stack.enter_context(tc.tile_pool(name="x04", bufs=1))
        h04 = [x04p.tile([P, E], f32r, tag=f"h04_{t}", name=f"h04_{t}")
               for t in range(4)]
        e04 = [x04p.tile([P, E], f32r, tag=f"e04_{t}", name=f"e04_{t}")
               for t in range(4)]

        load_norm(xn_d, h_sb, 0, 4, hfp, dst2=h04)
        load_norm(xe_d, e_sb, 0, 4, eearly, dst2=e04)
        transpose_local(h04, hTl)
        transpose_local(e04, eTl)
        x04_stack.close()

        # modules 1 and 5 need no aggregates (only hTl/eTl) - start DVE early
        module(1, eTl, eTl, xatt_h, first=True)
        module(5, eTl, eTl, xatt_e, first=True)

        load_norm(xn_d, h_sb, 4, NT, hfp)

        agg0_stack = ExitStack()
        agg0pool = agg0_stack.enter_context(tc.tile_pool(name="agg0p", bufs=1))
        agg0 = aggregate(0, h_sb, agg0pool)
        module(0, agg0, hTl, xatt_h, first=False)
        module(4, agg0, hTl, xatt_e, first=False)

        agg1 = aggregate(1, h_sb, agglate12)
        agg2 = aggregate(2, h_sb, agglate12)
        agg0_stack.close()
        hstack.close()

        module(2, eTl, agg1, xatt_h, first=False)

        # rest of e
        erest_stack = ExitStack()
        erest = erest_stack.enter_context(tc.tile_pool(name="erest", bufs=1))
        e_sb += [erest.tile([P, E], bf16, tag=f"e{t}", name=f"esb{t}")
                 for t in range(4, NT)]
        load_norm(xe_d, e_sb, 4, NT, erest)

        agg3 = aggregate(3, e_sb, agglate34)
        module(3, hTl, agg3, xatt_h, first=False)
        agg4 = aggregate(4, e_sb, agglate34)
        erest_stack.close()
        eearly_stack.close()
        ffn(xatt_h, w1h_d, w2h_d, b1h_t, b2h_t, outh_d)
        module(7, hTl, agg4, xatt_e, first=False)
        aggl34_stack.close()
        module(6, eTl, agg2, xatt_e, first=False)
        aggl12_stack.close()
        ffn(xatt_e, w1e_d, w2e_d, b1e_t, b2e_t, oute_d)

    _split_big_waits(nc, mybir)
    return nc


def _get_program():
    if "nc" not in _PROGRAM_CACHE:
        _PROGRAM_CACHE["nc"] = _build_program()
    return _PROGRAM_CACHE["nc"]


def _prep_inputs(x_node, x_edge, adj, Wq, Wk, Wv,
                 proj_he_h, proj_eh_h, proj_he_e, proj_eh_e,
                 rms1_h, rms1_e, rms2_h,
                 w1_h, b1_h, w2_h, b2_h, w1_e, b1_e, w2_e, b2_e):
    """Per-core input dicts. Weight folding + row rotation happen here."""
    f = np.float32
    wsrc_q = [rms1_h, rms1_e, rms1_e, rms1_h, rms1_h, rms1_e, rms1_e, rms1_h]
    wsrc_k = [rms1_h, rms1_e, rms1_h, rms1_e, rms1_h, rms1_e, rms1_h, rms1_e]
    wqT = np.stack([(Wq[m].T * wsrc_q[m][:, None]) * 0.125 for m in range(H)])
    wkT = np.stack([Wk[m].T * wsrc_k[m][:, None] for m in range(H)])
    wvT = np.stack([Wv[m].T * rms1_h[:, None] for m in range(H)])
    w1hT = np.ascontiguousarray((w1_h * rms2_h[None, :]).T, dtype=f)
    w1eT = np.ascontiguousarray((w1_e * rms2_h[None, :]).T, dtype=f)
    w2hT = np.ascontiguousarray(w2_h.T, dtype=f)
    w2eT = np.ascontiguousarray(w2_e.T, dtype=f)
    mats = [adj, proj_eh_h, proj_eh_e, proj_he_h, proj_he_e]

    shared = dict(wqT=np.ascontiguousarray(wqT, f), wkT=np.ascontiguousarray(wkT, f),
                  wvT=np.ascontiguousarray(wvT, f),
                  w1hT=w1hT, w2hT=w2hT, w1eT=w1eT, w2eT=w2eT,
                  b1h=b1_h.astype(f), b2h=b2_h.astype(f),
                  b1e=b1_e.astype(f), b2e=b2_e.astype(f))
    in_maps = []
    for c in range(NCORES):
        r0 = c * RPC
        m = dict(shared)
        m["xn"] = np.ascontiguousarray(np.roll(x_node, -r0, axis=0), f)
        m["xe"] = np.ascontiguousarray(np.roll(x_edge, -r0, axis=0), f)
        for i, mat in enumerate(mats):
            mt = np.ascontiguousarray(mat[r0:r0 + RPC].T, f)  # [N, RPC]
            m[f"mat{i}"] = np.ascontiguousarray(np.roll(mt, -r0, axis=0))
        in_maps.append(m)
    return in_maps


def kernel(**inputs):
    from concourse.bass_utils import run_bass_kernel_spmd
    nc = _get_program()
    in_maps = _prep_inputs(**{k: np.asarray(v) for k, v in inputs.items()})
    res = run_bass_kernel_spmd(nc, in_maps, list(range(NCORES))).results
    x_h = np.concatenate([res[c]["outh"] for c in range(NCORES)], axis=0)
    x_e = np.concatenate([res[c]["oute"] for c in range(NCORES)], axis=0)
    return (x_h, x_e)
